# revision 13
# baseline (speedup 1.0000x reference)
"""Causal attention kernel for Trainium2 (Bass/Tile), 8-core data-parallel.

Problem: x[32,1024,512] f32, W[512,1536] f32.
  kqv = x @ W; k,q,v = split(kqv); S = q k^T / sqrt(512) (causal);
  out = softmax(S) @ v.

Distribution: batch-parallel, 4 batches per core, weights replicated.

Per-core algorithm (per batch):
  - kT/qT ([C,T], C on partitions) via fp8 DoubleRow matmuls: host
    pre-interleaves x and W in contraction pairs ((p,j) <-> c=2p+j per
    128-pair chunk) and pre-permutes W columns so the kT/qT PSUM output
    partitions land directly in the pair-interleaved layout the scores
    matmul needs. W is pre-scaled by 32 to clear the fp8 subnormal range.
  - v ([T,C]) in float32r (full fp32 data, fast PE streaming mode).
  - Scores computed TRANSPOSED: ST[s,t] = k q^T via fp8 DoubleRow, so
    softmax normalization can be deferred: P^T = exp(ST*scale) (no
    max-subtraction: scores ~N(0,0.2), exp is safe), causal handled by
    skipping upper-triangle 128-blocks + one triangular mask multiply on
    the diagonal block.
  - out_raw = P^T v and row-sums via a parallel ones-column matmul, both
    in float32r; out = out_raw * (1/rowsum).
"""

import sys

sys.path.insert(0, "/opt/trn_rl_repo")

import numpy as np

import concourse.mybir as mybir
import concourse.tile as tile
from concourse import bacc
from concourse.bass_utils import run_bass_kernel_spmd

B, T, C = 32, 1024, 512
N_CORES = 8
BPC = B // N_CORES  # 4 batches per core
P = 128
NT = T // P  # 8 row tiles of T
NU = C // (2 * P)  # 2 pair-chunks of C (128 pairs each)
F32 = mybir.dt.float32
F32R = mybir.dt.float32r
BF16 = mybir.dt.bfloat16
FP8 = mybir.dt.float8e4
FP8E5 = mybir.dt.float8e5
EXP = mybir.ActivationFunctionType.Exp
DR = mybir.MatmulPerfMode.DoubleRow

W_SCALE = 32.0  # pre-scale for Wv in fp8 (clears subnormals)
M_SCALE = 64.0  # pre-scale for M = Wk Wq^T in fp8
SCORE_SCALE = float(C) ** -0.5 / M_SCALE

NP_FP8 = mybir.dt.np(FP8)
NP_FP8E5 = mybir.dt.np(FP8E5)
NP_BF16 = mybir.dt.np(BF16)

_CACHE = {}


def build_bass(repeats=1):
    nc = bacc.Bacc(None, target_bir_lowering=False)
    # x8: pair-interleaved fp8 x^T: [BPC, u, p, j, t] <-> x[b, t, 256u+2p+j]
    x8_d = nc.declare_dram_parameter("x8", [BPC, NU, P, 2, T], FP8, isOutput=False)
    # xr8: e5m2 residual x - fp8(x), same pair-interleaved layout — V is
    # computed residual-compensated in fp8 DoubleRow:
    #   32 v = x8·(32Wv)8 + xr·(32Wv)8 + x8·(32Wv − (32Wv)8)
    xr8_d = nc.declare_dram_parameter("xr8", [BPC, NU, P, 2, T], FP8E5, isOutput=False)
    # m8: M^T where M = Wk Wq^T (precomputed host-side so scores need only
    # ONE on-chip projection G = M x^T instead of kT and qT):
    # pair-interleaved rows (d), column-permuted (c' blocks (u',j')), x64
    m8_d = nc.declare_dram_parameter("m8", [NU, P, 2, C], FP8, isOutput=False)
    # w8v: fp8(32 Wv), pair-interleaved rows; wr8v: e5m2 residual of it
    w8v_d = nc.declare_dram_parameter("w8v", [NU, P, 2, C], FP8, isOutput=False)
    wr8v_d = nc.declare_dram_parameter("wr8v", [NU, P, 2, C], FP8E5, isOutput=False)
    # triangular keep-mask for diagonal blocks (upper-tri incl diag), bf16
    mask_d = nc.declare_dram_parameter("mask", [P, P], BF16, isOutput=False)
    # [32,0,0,0] per partition: appended to v tiles so the softmax denominator
    # rides along the P^T v matmul as an extra column; 32 matches the 32v
    # scale of the compensated V so normalization cancels it for free
    vpad_d = nc.declare_dram_parameter("vpad", [P, 4], BF16, isOutput=False)
    # out shipped bf16 to halve HBM writeback; host converts to f32
    out_d = nc.declare_dram_parameter("out", [BPC, T, C], BF16, isOutput=True)

    with tile.TileContext(nc) as tc:
        with (
            tc.tile_pool(name="const", bufs=1) as constp,
            tc.tile_pool(name="x8", bufs=2) as x8p,
            tc.tile_pool(name="xt", bufs=2) as xtp,
            tc.tile_pool(name="kq", bufs=2) as kqp,
            tc.tile_pool(name="v", bufs=2) as vp,
            tc.tile_pool(name="pt", bufs=3) as ptp,
            tc.tile_pool(name="osb", bufs=4) as osbp,
            tc.tile_pool(name="rec", bufs=4) as recp,
            tc.tile_pool(name="ps", bufs=2, space="PSUM") as psp,
            tc.tile_pool(name="psv", bufs=2, space="PSUM") as psvp,
            tc.tile_pool(name="pso", bufs=2, space="PSUM") as psop,
        ):
            # m8/x8 interleaved per-u so the first G matmul (needs only u=0
            # tiles) can start as early as possible.
            m8t = [constp.tile([P, 2, C], FP8, tag=f"m8{u}", name=f"m8_{u}")
                   for u in range(NU)]
            w8vt = [constp.tile([P, 2, C], FP8, tag=f"w8v{u}", name=f"w8v_{u}")
                    for u in range(NU)]
            wr8vt = [constp.tile([P, 2, C], FP8E5, tag=f"wr8v{u}", name=f"wr8v_{u}")
                     for u in range(NU)]
            maskt = constp.tile([P, P], BF16, tag="mask")
            vpad = constp.tile([P, 4], BF16, tag="vpad")

            first = True
            for li, b in enumerate(
                [b for _ in range(repeats) for b in range(BPC)]
            ):
                x8s = []
                for u in range(NU):
                    t_ = x8p.tile([P, 2, T], FP8, tag=f"x8{u}")
                    if first:
                        nc.sync.dma_start(m8t[u][:], m8_d[u])
                    nc.sync.dma_start(t_[:], x8_d[b, u])
                    x8s.append(t_)
                if first:
                    first = False
                    for u in range(NU):
                        nc.sync.dma_start(w8vt[u][:], w8v_d[u])
                        nc.sync.dma_start(wr8vt[u][:], wr8v_d[u])
                    nc.sync.dma_start(maskt[:], mask_d[:])
                    nc.sync.dma_start(vpad[:], vpad_d[:])
                xr8s = []
                for u in range(NU):
                    t_ = xtp.tile([P, 2, T], FP8E5, tag=f"xr8{u}")
                    nc.sync.dma_start(t_[:], xr8_d[b, u])
                    xr8s.append(t_)

                # G = M x^T via fp8 DoubleRow. Output block bi=(u',j') covers
                # rows c' = 256u' + 2p + j' of G, written pair-interleaved
                # into g8t[u'][:, j', :] so ST can contract x8 against it.
                g8t = [
                    kqp.tile([P, 2, T], FP8, tag=f"g8{u}", name=f"g8_{b}_{u}")
                    for u in range(NU)
                ]
                # Projections, interleaved: G groups are copy-bound (fast
                # fp8-DR matmuls, ACT PSUM drains) while V groups are
                # PE+DVE — mixing them keeps PE, ACT and DVE all busy.
                vs = [None] * NT

                def emit_v(tj):
                    # V group: residual-compensated fp8 DR — psum accumulates
                    # 32v = x8·w8v + xr·w8v + x8·wrv
                    ps = psvp.tile([P, 512], F32, tag="psv", name=f"psv{b}_{tj}")
                    terms = []
                    for u in range(NU):
                        xs = x8s[u][:, :, tj * P : (tj + 1) * P]
                        xrs = xr8s[u][:, :, tj * P : (tj + 1) * P]
                        terms += [(xs, w8vt[u]), (xrs, w8vt[u]), (xs, wr8vt[u])]
                    for ti, (lhs, rhs) in enumerate(terms):
                        nc.tensor.matmul(
                            ps[:], lhs, rhs[:],
                            start=(ti == 0),
                            stop=(ti == len(terms) - 1),
                            perf_mode=DR,
                        )
                    sb = vp.tile([P, C + 4], BF16, tag=f"v{tj}", name=f"v_{b}_{tj}")
                    nc.vector.tensor_copy(sb[:, :C], ps[:])
                    if li < 2:
                        # vpad cols are identical every batch; the 2-buf ring
                        # means each physical buffer is written once (li 0/1)
                        nc.vector.tensor_copy(sb[:, C : C + 4], vpad[:])
                    vs[tj] = sb

                for bi in range(8):
                    up, jp, h = (bi // 2) // 2, (bi // 2) % 2, bi % 2
                    ps = psp.tile([P, 512], F32, tag="ps")
                    for u in range(NU):
                        nc.tensor.matmul(
                            ps[:],
                            m8t[u][:, :, (2 * up + jp) * P : (2 * up + jp + 1) * P],
                            x8s[u][:, :, h * 512 : (h + 1) * 512],
                            start=(u == 0),
                            stop=(u == NU - 1),
                            perf_mode=DR,
                        )
                    nc.gpsimd.tensor_copy(
                        g8t[up][:, jp, h * 512 : (h + 1) * 512], ps[:]
                    )
                    if li > 0:
                        emit_v(bi)

                # out[tj] = (sum_{i<=tj} PT_i^T v_i) / rowsum, rowsum riding
                # as v's appended ones column. The two halves land in one
                # 2-bank PSUM tile (cols 0:256 and 512:772) so one strided
                # tensor_scalar normalizes both. Emitted interleaved with the
                # ST/exp loop below: PE runs PV(si-1) while ACT exps ST(si).
                H = C // 2
                pts = []

                def emit_pv(tj):
                    # second half (carries the rowsum cols) first: its bank
                    # finishes mid-PV so the reciprocal overlaps the first-
                    # half matmul chain instead of trailing it.
                    ps_o = psop.tile([P, 2 * C], F32, tag="pso", name=f"pso{b}_{tj}")
                    for i in range(tj + 1):
                        st, sp = (i == 0), (i == tj)
                        lhs = pts[i][:, tj * P : (tj + 1) * P]
                        nc.tensor.matmul(
                            ps_o[:, C : C + H + 4], lhs, vs[i][:, H:],
                            start=st, stop=sp,
                        )
                    rec = recp.tile([P, 1], F32, tag="rec", name=f"rec{b}_{tj}")
                    nc.vector.reciprocal(rec[:], ps_o[:, C + H : C + H + 1])
                    for i in range(tj + 1):
                        st, sp = (i == 0), (i == tj)
                        lhs = pts[i][:, tj * P : (tj + 1) * P]
                        nc.tensor.matmul(
                            ps_o[:, :H], lhs, vs[i][:, :H], start=st, stop=sp
                        )
                    osb = osbp.tile([P, C], BF16, tag="osb", name=f"osb{b}_{tj}")
                    nc.gpsimd.tensor_scalar_mul(
                        osb[:].rearrange("p (u h) -> p u h", u=2),
                        ps_o[:].rearrange("p (u h) -> p u h", u=2)[:, :, :H],
                        rec[:],
                    )
                    nc.sync.dma_start(out_d[b, tj * P : (tj + 1) * P, :], osb[:])

                # P^T tiles: PT[s,t] = exp(scale' * (32k)·(32q)), causal.
                for si in range(NT):
                    lo = si * P
                    pt_t = ptp.tile([P, T], BF16, tag=f"pt{si}")
                    w_all = T - lo
                    if w_all > 512:
                        half = (w_all // 2 + 127) // 128 * 128
                        chunks = [(lo, lo + half), (lo + half, T)]
                    else:
                        chunks = [(lo, T)]
                    for t0, t1 in chunks:
                        w_ = t1 - t0
                        ps = psp.tile([P, 512], F32, tag="ps")
                        for u in range(NU):
                            nc.tensor.matmul(
                                ps[:, :w_],
                                x8s[u][:, :, lo : lo + P],
                                g8t[u][:, :, t0:t1],
                                start=(u == 0),
                                stop=(u == NU - 1),
                                perf_mode=DR,
                            )
                        nc.scalar.activation(
                            pt_t[:, t0:t1], ps[:, :w_], EXP, scale=SCORE_SCALE
                        )
                    nc.gpsimd.tensor_mul(
                        pt_t[:, lo : lo + P], pt_t[:, lo : lo + P], maskt[:]
                    )
                    pts.append(pt_t)
                    if li > 0 and si >= 1:
                        emit_pv(si - 1)
                if li == 0:
                    # cold-start batch: V-path inputs (w8v/xr8) arrive last,
                    # so V/PV are emitted after ST to keep PE fed during the
                    # DMA ramp-in.
                    for tj in range(NT):
                        emit_v(tj)
                    for tj in range(NT - 1):
                        emit_pv(tj)
                emit_pv(NT - 1)

    nc.compile()
    return nc


def prep_inputs(x: np.ndarray, W_attn: np.ndarray):
    """Host-side sharding + layout transforms. Returns in_maps for 8 cores."""
    xt = np.ascontiguousarray(np.transpose(x, (0, 2, 1)))  # [B, C, T] f32
    # pair-interleaved fp8 x^T: [B, NU, P, 2, T], plus e5m2 residual
    xp = np.ascontiguousarray(xt.reshape(B, NU, P, 2, T))
    x8 = xp.astype(NP_FP8)
    xr8 = (xp - x8.astype(np.float32)).astype(NP_FP8E5)

    # M = Wk Wq^T precomputed host-side; shipped as M^T (contraction d on
    # rows), pair-interleaved rows, columns c' permuted into (u',j') blocks.
    wk, wq = W_attn[:, :C], W_attn[:, C : 2 * C]
    mt = (wk @ wq.T).T * M_SCALE  # [d, c']
    cols = []
    for up in range(2):
        for jp in range(2):
            cols.append(256 * up + jp + 2 * np.arange(P))
    colperm = np.concatenate(cols)
    m8 = mt[:, colperm].reshape(NU, P, 2, C).astype(NP_FP8)
    # Wv: 32x-scaled fp8 + e5m2 residual, pair-interleaved rows
    wv32 = np.ascontiguousarray(W_attn[:, 2 * C :] * W_SCALE).reshape(NU, P, 2, C)
    w8v = wv32.astype(NP_FP8)
    wr8v = (wv32 - w8v.astype(np.float32)).astype(NP_FP8E5)

    mask = np.triu(np.ones((P, P), dtype=np.float32)).astype(NP_BF16)
    vpad = np.zeros((P, 4), dtype=np.float32)
    vpad[:, 0] = W_SCALE  # matches the 32v scale; normalization cancels it
    vpad = vpad.astype(NP_BF16)

    in_maps = []
    for c in range(N_CORES):
        sl = slice(c * BPC, (c + 1) * BPC)
        in_maps.append(
            {
                "x8": x8[sl],
                "xr8": xr8[sl],
                "m8": m8,
                "w8v": w8v,
                "wr8v": wr8v,
                "mask": mask,
                "vpad": vpad,
            }
        )
    return in_maps


def kernel(x: np.ndarray, W_attn: np.ndarray) -> np.ndarray:
    x = np.asarray(x, dtype=np.float32)
    W_attn = np.asarray(W_attn, dtype=np.float32)
    if "nc" not in _CACHE:
        _CACHE["nc"] = build_bass()
    nc = _CACHE["nc"]
    in_maps = prep_inputs(x, W_attn)
    res = run_bass_kernel_spmd(nc, in_maps, list(range(N_CORES)))
    out = np.concatenate(
        [res.results[c]["out"].astype(np.float32) for c in range(N_CORES)], axis=0
    )
    return out



# revision 16
# speedup vs baseline: 1.0634x; 1.0634x over previous
"""Causal attention kernel for Trainium2 (Bass/Tile), 8-core data-parallel.

Problem: x[32,1024,512] f32, W[512,1536] f32.
  kqv = x @ W; k,q,v = split(kqv); S = q k^T / sqrt(512) (causal);
  out = softmax(S) @ v.

Distribution: batch-parallel, 4 batches per core, weights replicated.

Per-core algorithm (per batch):
  - kT/qT ([C,T], C on partitions) via fp8 DoubleRow matmuls: host
    pre-interleaves x and W in contraction pairs ((p,j) <-> c=2p+j per
    128-pair chunk) and pre-permutes W columns so the kT/qT PSUM output
    partitions land directly in the pair-interleaved layout the scores
    matmul needs. W is pre-scaled by 32 to clear the fp8 subnormal range.
  - v ([T,C]) in float32r (full fp32 data, fast PE streaming mode).
  - Scores computed TRANSPOSED: ST[s,t] = k q^T via fp8 DoubleRow, so
    softmax normalization can be deferred: P^T = exp(ST*scale) (no
    max-subtraction: scores ~N(0,0.2), exp is safe), causal handled by
    skipping upper-triangle 128-blocks + one triangular mask multiply on
    the diagonal block.
  - out_raw = P^T v and row-sums via a parallel ones-column matmul, both
    in float32r; out = out_raw * (1/rowsum).
"""

import sys

sys.path.insert(0, "/opt/trn_rl_repo")

import numpy as np

import concourse.mybir as mybir
import concourse.tile as tile
from concourse import bacc
from concourse.bass_utils import run_bass_kernel_spmd

B, T, C = 32, 1024, 512
N_CORES = 8
BPC = B // N_CORES  # 4 batches per core
P = 128
NT = T // P  # 8 row tiles of T
NU = C // (2 * P)  # 2 pair-chunks of C (128 pairs each)
F32 = mybir.dt.float32
F32R = mybir.dt.float32r
BF16 = mybir.dt.bfloat16
FP8 = mybir.dt.float8e4
FP8E5 = mybir.dt.float8e5
EXP = mybir.ActivationFunctionType.Exp
DR = mybir.MatmulPerfMode.DoubleRow

W_SCALE = 32.0  # pre-scale for Wv in fp8 (clears subnormals)
M_SCALE = 64.0  # pre-scale for M = Wk Wq^T in fp8
SCORE_SCALE = float(C) ** -0.5 / M_SCALE

NP_FP8 = mybir.dt.np(FP8)
NP_FP8E5 = mybir.dt.np(FP8E5)
NP_BF16 = mybir.dt.np(BF16)

_CACHE = {}


def build_bass(repeats=1):
    nc = bacc.Bacc(None, target_bir_lowering=False)
    # x8: pair-interleaved fp8 x^T: [BPC, u, p, j, t] <-> x[b, t, 256u+2p+j]
    x8_d = nc.declare_dram_parameter("x8", [BPC, NU, P, 2, T], FP8, isOutput=False)
    # xr8: e5m2 residual x - fp8(x), same pair-interleaved layout — V is
    # computed residual-compensated in fp8 DoubleRow:
    #   32 v = x8·(32Wv)8 + xr·(32Wv)8 + x8·(32Wv − (32Wv)8)
    xr8_d = nc.declare_dram_parameter("xr8", [BPC, NU, P, 2, T], FP8E5, isOutput=False)
    # m8: M^T where M = Wk Wq^T (precomputed host-side so scores need only
    # ONE on-chip projection G = M x^T instead of kT and qT):
    # pair-interleaved rows (d), column-permuted (c' blocks (u',j')), x64
    m8_d = nc.declare_dram_parameter("m8", [NU, P, 2, C], FP8, isOutput=False)
    # w8v: fp8(32 Wv), pair-interleaved rows; wr8v: e5m2 residual of it
    w8v_d = nc.declare_dram_parameter("w8v", [NU, P, 2, C], FP8, isOutput=False)
    wr8v_d = nc.declare_dram_parameter("wr8v", [NU, P, 2, C], FP8E5, isOutput=False)
    # triangular keep-mask for diagonal blocks (upper-tri incl diag), bf16
    mask_d = nc.declare_dram_parameter("mask", [P, P], BF16, isOutput=False)
    # [32,0,0,0] per partition: appended to v tiles so the softmax denominator
    # rides along the P^T v matmul as an extra column; 32 matches the 32v
    # scale of the compensated V so normalization cancels it for free
    vpad_d = nc.declare_dram_parameter("vpad", [P, 4], BF16, isOutput=False)
    # out shipped bf16 to halve HBM writeback; host converts to f32
    out_d = nc.declare_dram_parameter("out", [BPC, T, C], BF16, isOutput=True)

    with tile.TileContext(nc) as tc:
        with (
            tc.tile_pool(name="const", bufs=1) as constp,
            tc.tile_pool(name="x8", bufs=2) as x8p,
            tc.tile_pool(name="xt", bufs=2) as xtp,
            tc.tile_pool(name="kq", bufs=2) as kqp,
            tc.tile_pool(name="v", bufs=2) as vp,
            tc.tile_pool(name="pt", bufs=3) as ptp,
            tc.tile_pool(name="osb", bufs=4) as osbp,
            tc.tile_pool(name="rec", bufs=4) as recp,
            tc.tile_pool(name="ps", bufs=2, space="PSUM") as psp,
            tc.tile_pool(name="psv", bufs=2, space="PSUM") as psvp,
            tc.tile_pool(name="pso", bufs=2, space="PSUM") as psop,
        ):
            # m8/x8 interleaved per-u so the first G matmul (needs only u=0
            # tiles) can start as early as possible.
            m8t = [constp.tile([P, 2, C], FP8, tag=f"m8{u}", name=f"m8_{u}")
                   for u in range(NU)]
            w8vt = [constp.tile([P, 2, C], FP8, tag=f"w8v{u}", name=f"w8v_{u}")
                    for u in range(NU)]
            wr8vt = [constp.tile([P, 2, C], FP8E5, tag=f"wr8v{u}", name=f"wr8v_{u}")
                     for u in range(NU)]
            maskt = constp.tile([P, P], BF16, tag="mask")
            vpad = constp.tile([P, 4], BF16, tag="vpad")

            first = True
            for li, b in enumerate(
                [b for _ in range(repeats) for b in range(BPC)]
            ):
                x8s = []
                for u in range(NU):
                    t_ = x8p.tile([P, 2, T], FP8, tag=f"x8{u}")
                    if first:
                        nc.sync.dma_start(m8t[u][:], m8_d[u])
                    nc.sync.dma_start(t_[:], x8_d[b, u])
                    x8s.append(t_)
                if first:
                    first = False
                    for u in range(NU):
                        nc.sync.dma_start(w8vt[u][:], w8v_d[u])
                        nc.sync.dma_start(wr8vt[u][:], wr8v_d[u])
                    nc.sync.dma_start(maskt[:], mask_d[:])
                    nc.sync.dma_start(vpad[:], vpad_d[:])
                xr8s = []
                for u in range(NU):
                    t_ = xtp.tile([P, 2, T], FP8E5, tag=f"xr8{u}")
                    nc.sync.dma_start(t_[:], xr8_d[b, u])
                    xr8s.append(t_)

                # G = M x^T via fp8 DoubleRow. Output block bi=(u',j') covers
                # rows c' = 256u' + 2p + j' of G, written pair-interleaved
                # into g8t[u'][:, j', :] so ST can contract x8 against it.
                g8t = [
                    kqp.tile([P, 2, T], FP8, tag=f"g8{u}", name=f"g8_{b}_{u}")
                    for u in range(NU)
                ]
                # Projections, interleaved: G groups are copy-bound (fast
                # fp8-DR matmuls, ACT PSUM drains) while V groups are
                # PE+DVE — mixing them keeps PE, ACT and DVE all busy.
                vs = [None] * NT

                def emit_v(tj):
                    # V group: residual-compensated fp8 DR — psum accumulates
                    # 32v = x8·w8v + xr·w8v + x8·wrv
                    ps = psvp.tile([P, 512], F32, tag="psv", name=f"psv{b}_{tj}")
                    terms = []
                    for u in range(NU):
                        xs = x8s[u][:, :, tj * P : (tj + 1) * P]
                        xrs = xr8s[u][:, :, tj * P : (tj + 1) * P]
                        terms += [(xs, w8vt[u]), (xrs, w8vt[u]), (xs, wr8vt[u])]
                    for ti, (lhs, rhs) in enumerate(terms):
                        nc.tensor.matmul(
                            ps[:], lhs, rhs[:],
                            start=(ti == 0),
                            stop=(ti == len(terms) - 1),
                            perf_mode=DR,
                        )
                    sb = vp.tile([P, C + 4], BF16, tag=f"v{tj}", name=f"v_{b}_{tj}")
                    nc.vector.tensor_copy(sb[:, :C], ps[:])
                    if li < 2:
                        # vpad cols are identical every batch; the 2-buf ring
                        # means each physical buffer is written once (li 0/1)
                        nc.vector.tensor_copy(sb[:, C : C + 4], vpad[:])
                    vs[tj] = sb

                for bi in range(8):
                    up, jp, h = (bi // 2) // 2, (bi // 2) % 2, bi % 2
                    ps = psp.tile([P, 512], F32, tag="ps")
                    for u in range(NU):
                        nc.tensor.matmul(
                            ps[:],
                            m8t[u][:, :, (2 * up + jp) * P : (2 * up + jp + 1) * P],
                            x8s[u][:, :, h * 512 : (h + 1) * 512],
                            start=(u == 0),
                            stop=(u == NU - 1),
                            perf_mode=DR,
                        )
                    nc.scalar.copy(g8t[up][:, jp, h * 512 : (h + 1) * 512], ps[:])
                    if li > 0:
                        emit_v(bi)

                # out[tj] = (sum_{i<=tj} PT_i^T v_i) / rowsum, rowsum riding
                # as v's appended ones column. The two halves land in one
                # 2-bank PSUM tile (cols 0:256 and 512:772) so one strided
                # tensor_scalar normalizes both. Emitted interleaved with the
                # ST/exp loop below: PE runs PV(si-1) while ACT exps ST(si).
                H = C // 2
                pts = []

                def emit_pv(tj):
                    # second half (carries the rowsum cols) first: its bank
                    # finishes mid-PV so the reciprocal overlaps the first-
                    # half matmul chain instead of trailing it.
                    ps_o = psop.tile([P, 2 * C], F32, tag="pso", name=f"pso{b}_{tj}")
                    for i in range(tj + 1):
                        st, sp = (i == 0), (i == tj)
                        lhs = pts[i][:, tj * P : (tj + 1) * P]
                        nc.tensor.matmul(
                            ps_o[:, :H], lhs, vs[i][:, :H], start=st, stop=sp
                        )
                        nc.tensor.matmul(
                            ps_o[:, C : C + H + 4], lhs, vs[i][:, H:],
                            start=st, stop=sp,
                        )
                    rec = recp.tile([P, 1], F32, tag="rec", name=f"rec{b}_{tj}")
                    nc.vector.reciprocal(rec[:], ps_o[:, C + H : C + H + 1])
                    osb = osbp.tile([P, C], BF16, tag="osb", name=f"osb{b}_{tj}")
                    nc.vector.tensor_scalar_mul(
                        osb[:].rearrange("p (u h) -> p u h", u=2),
                        ps_o[:].rearrange("p (u h) -> p u h", u=2)[:, :, :H],
                        rec[:],
                    )
                    nc.sync.dma_start(out_d[b, tj * P : (tj + 1) * P, :], osb[:])

                # P^T tiles: PT[s,t] = exp(scale' * (32k)·(32q)), causal.
                for si in range(NT):
                    lo = si * P
                    pt_t = ptp.tile([P, T], BF16, tag=f"pt{si}")
                    w_all = T - lo
                    if w_all > 512:
                        half = (w_all // 2 + 127) // 128 * 128
                        chunks = [(lo, lo + half), (lo + half, T)]
                    else:
                        chunks = [(lo, T)]
                    for t0, t1 in chunks:
                        w_ = t1 - t0
                        ps = psp.tile([P, 512], F32, tag="ps")
                        for u in range(NU):
                            nc.tensor.matmul(
                                ps[:, :w_],
                                x8s[u][:, :, lo : lo + P],
                                g8t[u][:, :, t0:t1],
                                start=(u == 0),
                                stop=(u == NU - 1),
                                perf_mode=DR,
                            )
                        nc.scalar.activation(
                            pt_t[:, t0:t1], ps[:, :w_], EXP, scale=SCORE_SCALE
                        )
                    nc.vector.tensor_mul(
                        pt_t[:, lo : lo + P], pt_t[:, lo : lo + P], maskt[:]
                    )
                    pts.append(pt_t)
                    if li > 0 and si >= 1:
                        emit_pv(si - 1)
                if li == 0:
                    # cold-start batch: V-path inputs (w8v/xr8) arrive last,
                    # so V/PV are emitted after ST to keep PE fed during the
                    # DMA ramp-in.
                    for tj in range(NT):
                        emit_v(tj)
                    for tj in range(NT - 1):
                        emit_pv(tj)
                emit_pv(NT - 1)

    nc.compile()
    return nc


def prep_inputs(x: np.ndarray, W_attn: np.ndarray):
    """Host-side sharding + layout transforms. Returns in_maps for 8 cores."""
    xt = np.ascontiguousarray(np.transpose(x, (0, 2, 1)))  # [B, C, T] f32
    # pair-interleaved fp8 x^T: [B, NU, P, 2, T], plus e5m2 residual
    xp = np.ascontiguousarray(xt.reshape(B, NU, P, 2, T))
    x8 = xp.astype(NP_FP8)
    xr8 = (xp - x8.astype(np.float32)).astype(NP_FP8E5)

    # M = Wk Wq^T precomputed host-side; shipped as M^T (contraction d on
    # rows), pair-interleaved rows, columns c' permuted into (u',j') blocks.
    wk, wq = W_attn[:, :C], W_attn[:, C : 2 * C]
    mt = (wk @ wq.T).T * M_SCALE  # [d, c']
    cols = []
    for up in range(2):
        for jp in range(2):
            cols.append(256 * up + jp + 2 * np.arange(P))
    colperm = np.concatenate(cols)
    m8 = mt[:, colperm].reshape(NU, P, 2, C).astype(NP_FP8)
    # Wv: 32x-scaled fp8 + e5m2 residual, pair-interleaved rows
    wv32 = np.ascontiguousarray(W_attn[:, 2 * C :] * W_SCALE).reshape(NU, P, 2, C)
    w8v = wv32.astype(NP_FP8)
    wr8v = (wv32 - w8v.astype(np.float32)).astype(NP_FP8E5)

    mask = np.triu(np.ones((P, P), dtype=np.float32)).astype(NP_BF16)
    vpad = np.zeros((P, 4), dtype=np.float32)
    vpad[:, 0] = W_SCALE  # matches the 32v scale; normalization cancels it
    vpad = vpad.astype(NP_BF16)

    in_maps = []
    for c in range(N_CORES):
        sl = slice(c * BPC, (c + 1) * BPC)
        in_maps.append(
            {
                "x8": x8[sl],
                "xr8": xr8[sl],
                "m8": m8,
                "w8v": w8v,
                "wr8v": wr8v,
                "mask": mask,
                "vpad": vpad,
            }
        )
    return in_maps


def kernel(x: np.ndarray, W_attn: np.ndarray) -> np.ndarray:
    x = np.asarray(x, dtype=np.float32)
    W_attn = np.asarray(W_attn, dtype=np.float32)
    if "nc" not in _CACHE:
        _CACHE["nc"] = build_bass()
    nc = _CACHE["nc"]
    in_maps = prep_inputs(x, W_attn)
    res = run_bass_kernel_spmd(nc, in_maps, list(range(N_CORES)))
    out = np.concatenate(
        [res.results[c]["out"].astype(np.float32) for c in range(N_CORES)], axis=0
    )
    return out

